# revision 1
# baseline (speedup 1.0000x reference)
"""GATv3Conv Trainium2 kernel (8 NeuronCores, SPMD).

Strategy:
  - Shard EDGES by destination-node slice (core k owns dst in [k*6250,(k+1)*6250)).
    Segment softmax + aggregation are then fully core-local (no collectives).
  - Each core redundantly computes LayerNorm + the src/val GEMMs for ALL nodes
    (tables written to its HBM as one interleaved FSV table [N, 512] = [fs|fv]),
    and the dst GEMM only for its own slice (kept in SBUF).
  - Edge phase: edges host-sorted by dst window (128-node windows), padded to a
    fixed chunks-per-window schedule (SPMD-uniform). Per 128-edge chunk:
      fsv rows gathered from HBM via gpsimd.dma_gather (int16 idx, low/high
      table-half split), fd rows delivered via one-hot matmul from SBUF,
      e = silu(fs+fd) on ACT, score = per-head reduce of e*attn; per-window a
      single Exp over all chunk scores (avoids ACT table thrash); a one-hot
      matmul aggregates [msg | exp] into a PSUM accumulator per window.
  - Softmax division deferred to the end (commutes with the segment sum):
    out = silu(num/denom + h). exp() uses raw scores (no segment max):
    scores are O(+-10) here; mathematically identical to the reference.
  - All matmuls in float32r (full-rate PE, ~1e-4 rounding).
"""

import numpy as np

N_NODES = 50000
IN_FEATS = 256
OUT_FEATS = 256
NUM_HEADS = 8
FPH = OUT_FEATS // NUM_HEADS   # 32
LN_EPS = 1e-5
N_CORES = 8
SLICE = N_NODES // N_CORES     # 6250
P = 128
NWIN = (SLICE + P - 1) // P    # 49
SLICE_PAD = NWIN * P           # 6272
N_PAD = ((N_NODES + P - 1) // P) * P   # 50048
NTILES = N_PAD // P            # 391
HALF = 32768
FSV_COLS = 2 * OUT_FEATS       # 512
AGG_COLS = OUT_FEATS + NUM_HEADS  # 264
MAX_CALL_CHUNKS = 8            # <=1024 idx per dma_gather call
TABLE_BF16 = False              # FSV table in bf16 (halves gather/write bytes)

_CACHE = {}


def _build_nc(lowC, highC, n_nodes_pad=None, sim_safe=False, phases="ABC", reps=1):
    if n_nodes_pad is None:
        n_nodes_pad = N_PAD
    import concourse.bacc as bacc
    import concourse.tile as tile
    from concourse import mybir
    from contextlib import ExitStack

    f32 = mybir.dt.float32
    f32r = mybir.dt.float32r
    bf16 = mybir.dt.bfloat16
    tdt = bf16 if TABLE_BF16 else f32r
    AF = mybir.ActivationFunctionType
    CPW = lowC + highC

    def _split(n):
        out = []
        while n > 0:
            g = min(n, MAX_CALL_CHUNKS)
            out.append(g)
            n -= g
        return out

    call_plan = [(g, False) for g in _split(lowC)] + [(g, True) for g in _split(highC)]
    idx_cols_per_win = sum(g * P // 16 for g, _ in call_plan)

    nc = bacc.Bacc(None, target_bir_lowering=False)

    feat_t = nc.dram_tensor("feat", [n_nodes_pad, IN_FEATS], f32, kind="ExternalInput")
    featmy_t = nc.dram_tensor("featmy", [SLICE_PAD, IN_FEATS], f32, kind="ExternalInput")
    wfsv_t = nc.dram_tensor("wfsv", [P, 2, FSV_COLS], f32r, kind="ExternalInput")
    wfd_t = nc.dram_tensor("wfd", [P, 2, OUT_FEATS], f32r, kind="ExternalInput")
    attn_t = nc.dram_tensor("attnr", [P, OUT_FEATS], f32, kind="ExternalInput")
    ident_t = nc.dram_tensor("ident", [P, P], f32r, kind="ExternalInput")
    identb_t = nc.dram_tensor("identb", [P, P], mybir.dt.bfloat16, kind="ExternalInput")
    iota_t = nc.dram_tensor("iotar", [P, P], f32, kind="ExternalInput")
    dstf_t = nc.dram_tensor("dstf", [P, NWIN * CPW], f32, kind="ExternalInput")
    src32_t = nc.dram_tensor("src32", [P, NWIN * CPW], mybir.dt.int32, kind="ExternalInput")
    gidx_t = nc.dram_tensor("gidx", [P, NWIN * idx_cols_per_win], mybir.dt.int16,
                            kind="ExternalInput")
    out_t = nc.dram_tensor("outmy", [SLICE_PAD, OUT_FEATS], f32, kind="ExternalOutput")

    fsv_t = nc.dram_tensor("fsvtbl", [n_nodes_pad, FSV_COLS], tdt, kind="Internal")
    hupd_t = nc.dram_tensor("hupdtbl", [NWIN, P, AGG_COLS], f32, kind="Internal")

    with tile.TileContext(nc) as tc, ExitStack() as ctx:
        if reps > 1:
            ctx.enter_context(tc.For_i(0, reps, 1))
        const = ctx.enter_context(tc.tile_pool(name="const", bufs=1))
        persist = ctx.enter_context(tc.tile_pool(name="persist", bufs=1))

        def act_silu(out, in_, sbp, tag):
            if not sim_safe:
                nc.scalar.activation(out=out, in_=in_, func=AF.Silu)
            else:
                sg = sbp.tile(list(out.shape), f32, tag=tag)
                nc.scalar.activation(out=sg[:], in_=in_, func=AF.Sigmoid)
                nc.vector.tensor_mul(out=out, in0=in_, in1=sg[:])

        wfsv = const.tile([P, 2, FSV_COLS], f32r)
        nc.sync.dma_start(out=wfsv, in_=wfsv_t[:, :, :])
        wfd = const.tile([P, 2, OUT_FEATS], f32r)
        nc.sync.dma_start(out=wfd, in_=wfd_t[:, :, :])
        attn = const.tile([P, OUT_FEATS], f32)
        nc.sync.dma_start(out=attn, in_=attn_t[:, :])
        ident = const.tile([P, P], f32r)
        nc.sync.dma_start(out=ident, in_=ident_t[:, :])
        identb = const.tile([P, P], mybir.dt.bfloat16)
        nc.sync.dma_start(out=identb, in_=identb_t[:, :])
        iota = const.tile([P, P], f32)
        nc.sync.dma_start(out=iota, in_=iota_t[:, :])
        dstf = const.tile([P, NWIN * CPW], f32)
        nc.sync.dma_start(out=dstf, in_=dstf_t[:, :])
        src32 = const.tile([P, NWIN * CPW], mybir.dt.int32)
        nc.sync.dma_start(out=src32, in_=src32_t[:, :])
        gidx = const.tile([P, NWIN * idx_cols_per_win], mybir.dt.int16)
        nc.sync.dma_start(out=gidx, in_=gidx_t[:, :])
        eps_c = const.tile([P, 1], f32)
        nc.vector.memset(eps_c[:], LN_EPS)

        fd_slice = persist.tile([P, NWIN, OUT_FEATS], f32r)   # dst-proj of my slice
        stats_my = persist.tile([P, NWIN, 2], f32)            # (mean, rstd) of my slice

        # ---------------- Phase A: LN + GEMM tables ----------------
        def node_sweep(src_dram, ntiles, emit):
            with tc.tile_pool(name="a_sb", bufs=4) as sbp, \
                 tc.tile_pool(name="a_ps", bufs=2, space="PSUM") as psp:
                for t in range(ntiles):
                    F = sbp.tile([P, IN_FEATS], f32, tag="F")
                    nc.sync.dma_start(out=F, in_=src_dram[t * P:(t + 1) * P, :])
                    st = sbp.tile([P, 6], f32, tag="st")
                    nc.vector.bn_stats(out=st[:], in_=F[:])
                    mv = sbp.tile([P, 2], f32, tag="mv")
                    nc.vector.bn_aggr(out=mv[:], in_=st[:])
                    sd = sbp.tile([P, 1], f32, tag="sd")
                    nc.scalar.activation(out=sd[:], in_=mv[:, 1:2], func=AF.Sqrt,
                                         bias=eps_c[:])
                    rstd = sbp.tile([P, 1], f32, tag="rstd")
                    nc.vector.reciprocal(out=rstd[:], in_=sd[:])
                    h = sbp.tile([P, IN_FEATS], f32r, tag="h")
                    nc.vector.tensor_scalar(
                        out=h[:], in0=F[:], scalar1=mv[:, 0:1], scalar2=rstd[:],
                        op0=mybir.AluOpType.subtract, op1=mybir.AluOpType.mult)
                    tp = psp.tile([P, IN_FEATS], f32r, tag="tp")
                    nc.tensor.transpose(out=tp[:, 0:P], in_=h[:, 0:P],
                                        identity=ident[:])
                    nc.tensor.transpose(out=tp[:, P:2 * P], in_=h[:, P:2 * P],
                                        identity=ident[:])
                    hT = sbp.tile([P, 2, P], f32r, tag="hT")
                    nc.vector.tensor_copy(out=hT[:, 0, :], in_=tp[:, 0:P])
                    nc.scalar.activation(out=hT[:, 1, :], in_=tp[:, P:2 * P],
                                         func=AF.Copy)
                    emit(t, hT, mv, rstd, sbp, psp)

        def emit_fsv(t, hT, mv, rstd, sbp, psp):
            g = psp.tile([P, FSV_COLS], f32, tag="gemm")
            nc.tensor.matmul(out=g[:], lhsT=hT[:, 0, :], rhs=wfsv[:, 0, :],
                             start=True, stop=False)
            nc.tensor.matmul(out=g[:], lhsT=hT[:, 1, :], rhs=wfsv[:, 1, :],
                             start=False, stop=True)
            fsv = sbp.tile([P, FSV_COLS], tdt, tag="fsv")
            nc.vector.tensor_copy(out=fsv[:, 0:OUT_FEATS], in_=g[:, 0:OUT_FEATS])
            nc.scalar.activation(out=fsv[:, OUT_FEATS:], in_=g[:, OUT_FEATS:],
                                 func=AF.Copy)
            nc.sync.dma_start(out=fsv_t[t * P:(t + 1) * P, :], in_=fsv[:])

        def emit_fd(t, hT, mv, rstd, sbp, psp):
            g = psp.tile([P, OUT_FEATS], f32, tag="gemm")
            nc.tensor.matmul(out=g[:], lhsT=hT[:, 0, :], rhs=wfd[:, 0, :],
                             start=True, stop=False)
            nc.tensor.matmul(out=g[:], lhsT=hT[:, 1, :], rhs=wfd[:, 1, :],
                             start=False, stop=True)
            nc.scalar.activation(out=fd_slice[:, t, :], in_=g[:], func=AF.Copy)
            nc.vector.tensor_copy(out=stats_my[:, t, 0:1], in_=mv[:, 0:1])
            nc.vector.tensor_copy(out=stats_my[:, t, 1:2], in_=rstd[:])

        if "A" in phases:
            node_sweep(feat_t, NTILES, emit_fsv)
        node_sweep(featmy_t, NWIN, emit_fd)

        # ---------------- Phase B: edge phase ----------------
        def _phase_b():
            fsv_hi = fsv_t[HALF:n_nodes_pad, :]
            import os as _os
            _bb = lambda k, d: int(_os.environ.get(k, d))
            with tc.tile_pool(name="b_g", bufs=len(call_plan) + _bb("BGX", 1)) as gp, \
                 tc.tile_pool(name="b_sb", bufs=_bb("BSB", 4)) as sbp, \
                 tc.tile_pool(name="b_oha", bufs=_bb("BOHA", 3)) as ohap, \
                 tc.tile_pool(name="b_s8", bufs=_bb("BS8", 2)) as s8p, \
                 tc.tile_pool(name="b_evec", bufs=_bb("BEV", 3), space="PSUM") as evp, \
                 tc.tile_pool(name="b_ohg", bufs=_bb("BOG", 2), space="PSUM") as ogp, \
                 tc.tile_pool(name="b_hupd", bufs=_bb("BHU", 2), space="PSUM") as hup:
                icol = 0
                for w in range(NWIN):
                    tiles = []
                    for g_chunks, is_high in call_plan:
                        ni = g_chunks * P
                        G = gp.tile([P, MAX_CALL_CHUNKS, FSV_COLS], tdt, tag="G")
                        nc.gpsimd.dma_gather(
                            out_ap=G[:, 0:g_chunks, :],
                            in_ap=(fsv_hi if is_high else fsv_t[:, :]),
                            idxs_ap=gidx[:, icol:icol + ni // 16],
                            num_idxs=ni, num_idxs_reg=ni, elem_size=FSV_COLS,
                        )
                        icol += ni // 16
                        tiles.append((G, g_chunks))

                    hupd = hup.tile([P, AGG_COLS], f32, tag="hupd")
                    s8 = s8p.tile([P, CPW, NUM_HEADS], f32, tag="s8")
                    e8 = s8p.tile([P, CPW, NUM_HEADS], f32, tag="e8")
                    oh_all = ohap.tile([P, CPW, P], f32r, tag="oh_a")
                    c_glob = w * CPW

                    oh_as = []
                    c = 0
                    for (G, g_chunks) in tiles:
                        for lc in range(g_chunks):
                            fs = G[:, lc, 0:OUT_FEATS]
                            dcol = dstf[:, c_glob + c:c_glob + c + 1]
                            oh_a = oh_all[:, c, :]
                            nc.vector.tensor_tensor(
                                out=oh_a, in0=dcol.to_broadcast([P, P]), in1=iota[:],
                                op=mybir.AluOpType.is_equal)
                            ogT = ogp.tile([P, P], f32r, tag="ogT")
                            nc.tensor.transpose(out=ogT[:], in_=oh_a, identity=ident[:])
                            oh_g = sbp.tile([P, P], f32r, tag="oh_g")
                            nc.scalar.activation(out=oh_g[:], in_=ogT[:], func=AF.Copy)
                            ev = evp.tile([P, OUT_FEATS], f32, tag="ev")
                            nc.tensor.matmul(out=ev[:], lhsT=oh_g[:],
                                             rhs=fd_slice[:, w, :], start=True, stop=False)
                            nc.tensor.matmul(out=ev[:],
                                             lhsT=(identb[:] if TABLE_BF16 else ident[:]),
                                             rhs=fs, start=False, stop=True)
                            sevec = sbp.tile([P, OUT_FEATS], f32, tag="sevec")
                            act_silu(sevec[:], ev[:], sbp, "sg_b")
                            smul = sbp.tile([P, OUT_FEATS], f32, tag="smul")
                            nc.vector.tensor_mul(out=smul[:], in0=sevec[:], in1=attn[:])
                            nc.vector.tensor_reduce(
                                out=s8[:, c, :],
                                in_=smul[:].rearrange("p (h f) -> p h f", h=NUM_HEADS),
                                axis=mybir.AxisListType.X, op=mybir.AluOpType.add)
                            oh_as.append((oh_a, G, lc))
                            c += 1
                    nc.scalar.activation(out=e8[:], in_=s8[:], func=AF.Exp)
                    for c, (oh_a, G, lc) in enumerate(oh_as):
                        fv = G[:, lc, OUT_FEATS:FSV_COLS]
                        md = sbp.tile([P, AGG_COLS], f32r, tag="md")
                        nc.scalar.activation(out=md[:, OUT_FEATS:], in_=e8[:, c, :],
                                             func=AF.Copy)
                        nc.vector.tensor_tensor(
                            out=md[:, 0:OUT_FEATS].rearrange(
                                "p (h f) -> p h f", h=NUM_HEADS),
                            in0=fv.rearrange("p (h f) -> p h f", h=NUM_HEADS),
                            in1=e8[:, c, :][:, :, None].to_broadcast(
                                [P, NUM_HEADS, FPH]),
                            op=mybir.AluOpType.mult)
                        nc.tensor.matmul(out=hupd[:], lhsT=oh_a, rhs=md[:],
                                         start=(c == 0), stop=(c == CPW - 1))
                    hw_s = sbp.tile([P, AGG_COLS], f32, tag="hw_s")
                    nc.scalar.activation(out=hw_s[:], in_=hupd[:], func=AF.Copy)
                    nc.sync.dma_start(out=hupd_t[w], in_=hw_s[:])

        if "B" in phases:
            _phase_b()

        def _phase_c():
            # ---------------- Phase C: normalize + residual + silu ----------------
            with tc.tile_pool(name="c_sb", bufs=4) as sbp:
                for w in range(NWIN):
                    F = sbp.tile([P, IN_FEATS], f32, tag="F")
                    nc.sync.dma_start(out=F, in_=featmy_t[w * P:(w + 1) * P, :])
                    h = sbp.tile([P, IN_FEATS], f32, tag="h")
                    nc.vector.tensor_scalar(
                        out=h[:], in0=F[:], scalar1=stats_my[:, w, 0:1],
                        scalar2=stats_my[:, w, 1:2],
                        op0=mybir.AluOpType.subtract, op1=mybir.AluOpType.mult)
                    hu = sbp.tile([P, AGG_COLS], f32, tag="hu")
                    nc.sync.dma_start(out=hu, in_=hupd_t[w])
                    den = sbp.tile([P, NUM_HEADS], f32, tag="den")
                    nc.vector.tensor_scalar_add(out=den[:], in0=hu[:, OUT_FEATS:],
                                                scalar1=1e-30)
                    denr = sbp.tile([P, NUM_HEADS], f32, tag="denr")
                    nc.vector.reciprocal(out=denr[:], in_=den[:])
                    o = sbp.tile([P, OUT_FEATS], f32, tag="o")
                    nc.vector.tensor_tensor(
                        out=o[:].rearrange("p (h f) -> p h f", h=NUM_HEADS),
                        in0=hu[:, 0:OUT_FEATS].rearrange(
                            "p (h f) -> p h f", h=NUM_HEADS),
                        in1=denr[:][:, :, None].to_broadcast([P, NUM_HEADS, FPH]),
                        op=mybir.AluOpType.mult)
                    nc.vector.tensor_add(out=o[:], in0=o[:], in1=h[:])
                    oo = sbp.tile([P, OUT_FEATS], f32, tag="oo")
                    act_silu(oo[:], o[:], sbp, "sg_c")
                    nc.sync.dma_start(out=out_t[w * P:(w + 1) * P, :], in_=oo[:])

        if "C" in phases:
            _phase_c()

    nc.compile()
    return nc, call_plan, CPW


def _prepare_core_inputs(core, inputs, lowC, highC, call_plan):
    """Host-side schedule for one core: edge slotting + gather idx layout."""
    src = np.asarray(inputs["src"], np.int64)
    dst = np.asarray(inputs["dst"], np.int64)
    CPW = lowC + highC

    lo, hi = core * SLICE, (core + 1) * SLICE
    sel = np.where((dst >= lo) & (dst < hi))[0]
    dsl = dst[sel] - lo
    ssl = src[sel]
    w_of = dsl // P

    n_slots = NWIN * CPW * P
    slot_src = np.zeros(n_slots, np.int64)
    slot_dstf = np.full(n_slots, -1.0, np.float32)

    order = np.argsort(w_of, kind="stable")
    dsl_o, ssl_o, w_o = dsl[order], ssl[order], w_of[order]
    starts = np.searchsorted(w_o, np.arange(NWIN))
    ends = np.searchsorted(w_o, np.arange(NWIN) + 1)
    for w in range(NWIN):
        es, ee = starts[w], ends[w]
        s_w, d_w = ssl_o[es:ee], dsl_o[es:ee]
        is_lo = s_w < HALF
        base = w * CPW * P
        for half, sec_off, sec_chunks in ((True, 0, lowC), (False, lowC * P, highC)):
            s_h = s_w[is_lo == half]
            d_h = d_w[is_lo == half]
            n = len(s_h)
            assert n <= sec_chunks * P, (core, w, half, n, sec_chunks * P)
            sl = slice(base + sec_off, base + sec_off + n)
            slot_src[sl] = s_h
            slot_dstf[sl] = (d_h - w * P).astype(np.float32)

    dstf = slot_dstf.reshape(NWIN * CPW, P).T.copy()
    src32 = np.maximum(slot_src, 0).reshape(NWIN * CPW, P).T.astype(np.int32).copy()

    idx_cols = []
    for w in range(NWIN):
        base = w * CPW * P
        off = 0
        for g_chunks, is_high in call_plan:
            ni = g_chunks * P
            s = slot_src[base + off: base + off + ni].copy()
            if is_high:
                s = s - HALF
                s[s < 0] = 0
            import os as _os4
            if _os4.environ.get("SEQIDX") == "1":
                s = (np.arange(ni, dtype=np.int64) + 17) % 32000
            idx = s.astype(np.int16)
            wrapped = np.tile(idx.reshape(ni // 16, 16).T, (8, 1))
            idx_cols.append(wrapped)
            off += ni
    gidx = np.concatenate(idx_cols, axis=1).astype(np.int16)
    return dstf, gidx, src32


def _derive_schedule(src, dst):
    lowCs, highCs = 1, 1
    for core in range(N_CORES):
        lo, hi = core * SLICE, (core + 1) * SLICE
        m = (dst >= lo) & (dst < hi)
        dsl = dst[m] - lo
        ssl = src[m]
        w_of = dsl // P
        for w in range(NWIN):
            wm = w_of == w
            nlo = int(np.sum(wm & (ssl < HALF)))
            nhi = int(np.sum(wm & (ssl >= HALF)))
            lowCs = max(lowCs, (nlo + P - 1) // P)
            highCs = max(highCs, (nhi + P - 1) // P)
    return lowCs, highCs


def _shared_inputs(inputs):
    feat = np.asarray(inputs["feat"], np.float32)
    Wsrc = np.asarray(inputs["Wsrc"], np.float32)
    Wdst = np.asarray(inputs["Wdst"], np.float32)
    Wval = np.asarray(inputs["Wval"], np.float32)
    attn = np.asarray(inputs["attn"], np.float32)
    feat_pad = np.zeros((N_PAD, IN_FEATS), np.float32)
    feat_pad[:N_NODES] = feat
    wfsv = np.zeros((P, 2, FSV_COLS), np.float32)
    wfsv[:, 0, 0:OUT_FEATS] = Wsrc.T[0:P, :]
    wfsv[:, 1, 0:OUT_FEATS] = Wsrc.T[P:2 * P, :]
    wfsv[:, 0, OUT_FEATS:] = Wval.T[0:P, :]
    wfsv[:, 1, OUT_FEATS:] = Wval.T[P:2 * P, :]
    wfd = np.zeros((P, 2, OUT_FEATS), np.float32)
    wfd[:, 0, :] = Wdst.T[0:P, :]
    wfd[:, 1, :] = Wdst.T[P:2 * P, :]
    attn_rep = np.tile(attn.reshape(1, OUT_FEATS), (P, 1)).astype(np.float32)
    ident = np.eye(P, dtype=np.float32)
    import ml_dtypes
    identb = np.eye(P, dtype=ml_dtypes.bfloat16)
    iota = np.tile(np.arange(P, dtype=np.float32).reshape(1, P), (P, 1))
    return feat, feat_pad, wfsv, wfd, attn_rep, ident, identb, iota


def make_in_maps(inputs, lowCs, highCs, call_plan):
    feat, feat_pad, wfsv, wfd, attn_rep, ident, identb, iota = _shared_inputs(inputs)
    in_maps = []
    for core in range(N_CORES):
        dstf, gidx, src32 = _prepare_core_inputs(core, inputs, lowCs, highCs, call_plan)
        featmy = np.zeros((SLICE_PAD, IN_FEATS), np.float32)
        featmy[:SLICE] = feat[core * SLICE:(core + 1) * SLICE]
        in_maps.append(dict(
            feat=feat_pad, featmy=featmy, wfsv=wfsv, wfd=wfd, attnr=attn_rep,
            ident=ident, identb=identb, iotar=iota, dstf=dstf, gidx=gidx,
            src32=src32,
        ))
    return in_maps


def kernel(**inputs):
    import concourse.bass_utils as bass_utils

    for b in ("bsrc", "bdst", "bval"):
        assert not np.any(np.asarray(inputs[b])), \
            "nonzero biases unsupported by this kernel"
    src = np.asarray(inputs["src"], np.int64)
    dst = np.asarray(inputs["dst"], np.int64)

    lowCs, highCs = _derive_schedule(src, dst)
    key = (lowCs, highCs)
    if key not in _CACHE:
        _CACHE[key] = _build_nc(lowCs, highCs)
    nc, call_plan, CPW = _CACHE[key]

    in_maps = make_in_maps(inputs, lowCs, highCs, call_plan)
    res = bass_utils.run_bass_kernel_spmd(nc, in_maps, core_ids=list(range(N_CORES)))
    out = np.concatenate(
        [res.results[c]["outmy"][:SLICE] for c in range(N_CORES)], axis=0)
    return np.ascontiguousarray(out.astype(np.float32))



# revision 7
# speedup vs baseline: 1.2588x; 1.2588x over previous
"""GATv3Conv Trainium2 kernel (8 NeuronCores, SPMD).

Strategy (v2):
  - Shard EDGES by destination-node slice (core k owns dst in [k*6250,(k+1)*6250)).
    Segment softmax + aggregation are fully core-local (no collectives).
  - Each core redundantly computes LayerNorm + the src/val GEMMs for ALL nodes
    into a bf16 table [N,512] = [fs|fv] in its HBM (features (f,h)-major so the
    per-edge exp-broadcast multiply hits the DVE 2x 16-bit path), and the dst
    GEMM only for its own slice (kept in SBUF as bf16).
  - Edge phase, per 128-dst-node window, edges in 128-edge chunks:
      * fsv rows gathered via gpsimd.dma_gather (1 call per table-half).
      * evT[f,e] = (one-hot dst gather of fd via matmul) + (fs^T via
        identity-moving matmul), accumulated in PSUM.
      * silu on ACT from PSUM (2-chunk groups).
      * score[e,h] via PE: lhsT=sevT half, rhs=block-diag attn [128,8] - the
        8-wide output makes these matmuls nearly free.
      * one Exp per window (scores [P,CPW*8]); windows processed in pairs with
        silu/exp lag so ACT table loads halve.
      * md = fv * e8 (DVE bf16 2x, (f,h)-major broadcast), aggregation +
        denominators via one-hot matmuls into a [P,264] PSUM accumulator.
  - Softmax division deferred to the end: out = silu(num/den + h). exp() uses
    raw scores (no segment max): scores are O(+-10); identical to reference.
"""

import numpy as np

N_NODES = 50000
IN_FEATS = 256
OUT_FEATS = 256
NUM_HEADS = 8
FPH = OUT_FEATS // NUM_HEADS   # 32
LN_EPS = 1e-5
N_CORES = 8
SLICE = N_NODES // N_CORES     # 6250
P = 128
NWIN = (SLICE + P - 1) // P    # 49
SLICE_PAD = NWIN * P           # 6272
N_PAD = ((N_NODES + P - 1) // P) * P   # 50048
NTILES = N_PAD // P            # 391
HALF = 32768
TBL_COLS = 2 * OUT_FEATS       # 512
AGG_COLS = OUT_FEATS + NUM_HEADS  # 264
ATILE = 4                      # node tiles per phase-A DMA batch

# new feature order is (f, h)-major: new col j=f*8+h <- old col h*32+f
_OLD_OF_NEW = (np.arange(OUT_FEATS) % NUM_HEADS) * FPH + \
    np.arange(OUT_FEATS) // NUM_HEADS

_CACHE = {}


def _build_nc(lowC, highC, reps=1):
    import concourse.bacc as bacc
    import concourse.tile as tile
    from concourse import mybir
    from contextlib import ExitStack

    f32 = mybir.dt.float32
    bf16 = mybir.dt.bfloat16
    i16 = mybir.dt.int16
    AF = mybir.ActivationFunctionType
    Alu = mybir.AluOpType

    lowC = list(lowC)
    highC = list(highC)
    cpw = [l + h for l, h in zip(lowC, highC)]
    cbase = np.concatenate([[0], np.cumsum(cpw)]).astype(int)
    C_TOT = int(cbase[-1])
    CPWMX = max(cpw)
    LCMX = max(lowC)
    HCMX = max(max(highC), 1)
    icols = 8 * C_TOT  # int16 idx cols (128 idx -> 8 cols of 16)

    nc = bacc.Bacc(None, target_bir_lowering=False)

    featb_t = nc.dram_tensor("featb", [N_PAD, IN_FEATS], bf16, kind="ExternalInput")
    featmy_t = nc.dram_tensor("featmy", [SLICE_PAD, IN_FEATS], bf16,
                              kind="ExternalInput")
    wfsv_t = nc.dram_tensor("wfsv", [P, 2, TBL_COLS], bf16, kind="ExternalInput")
    wfd_t = nc.dram_tensor("wfd", [P, 2, OUT_FEATS], bf16, kind="ExternalInput")
    attnb_t = nc.dram_tensor("attnb", [P, 2, NUM_HEADS], bf16, kind="ExternalInput")
    identb_t = nc.dram_tensor("identb", [P, P], bf16, kind="ExternalInput")
    iotab_t = nc.dram_tensor("iotab", [P, P], bf16, kind="ExternalInput")
    dstf_t = nc.dram_tensor("dstf", [P, C_TOT], f32, kind="ExternalInput")
    gidx_t = nc.dram_tensor("gidx", [P, icols], i16, kind="ExternalInput")
    ohg_t = nc.dram_tensor("ohg", [P, C_TOT, P], bf16, kind="ExternalInput")
    out_t = nc.dram_tensor("outmy", [SLICE_PAD, OUT_FEATS], f32,
                           kind="ExternalOutput")

    fsv_t = nc.dram_tensor("fsvtbl", [N_PAD, TBL_COLS], bf16, kind="Internal")
    hupd_t = nc.dram_tensor("hupdtbl", [NWIN, P, AGG_COLS], bf16, kind="Internal")

    with tile.TileContext(nc) as tc, ExitStack() as ctx:
        if reps > 1:
            ctx.enter_context(tc.For_i(0, reps, 1))
        const = ctx.enter_context(tc.tile_pool(name="const", bufs=1))
        persist = ctx.enter_context(tc.tile_pool(name="persist", bufs=1))

        wfsv = const.tile([P, 2, TBL_COLS], bf16)
        nc.sync.dma_start(out=wfsv, in_=wfsv_t[:, :, :])
        wfd = const.tile([P, 2, OUT_FEATS], bf16)
        nc.sync.dma_start(out=wfd, in_=wfd_t[:, :, :])
        attnb = const.tile([P, 2, NUM_HEADS], bf16)
        nc.sync.dma_start(out=attnb, in_=attnb_t[:, :, :])
        identb = const.tile([P, P], bf16)
        nc.sync.dma_start(out=identb, in_=identb_t[:, :])
        iotab = const.tile([P, P], bf16)
        nc.sync.dma_start(out=iotab, in_=iotab_t[:, :])
        dstf = const.tile([P, C_TOT], f32)
        nc.sync.dma_start(out=dstf, in_=dstf_t[:, :])
        gidx = const.tile([P, icols], i16)
        nc.sync.dma_start(out=gidx, in_=gidx_t[:, :])
        eps_c = const.tile([P, 1], f32)
        nc.vector.memset(eps_c[:], LN_EPS)

        fd_slice = persist.tile([P, NWIN, OUT_FEATS], bf16)
        stats_my = persist.tile([P, NWIN, 2], f32)   # (mean, rstd)

        # ---------------- Phase A: LN + GEMM tables ----------------
        def node_sweep(src_dram, ntiles, emit, wtile, wcols):
            with tc.tile_pool(name="a_sb", bufs=3) as sbp, \
                 tc.tile_pool(name="a_ps", bufs=2, space="PSUM") as psp:
                for t0 in range(0, ntiles, ATILE):
                    bt = min(ATILE, ntiles - t0)
                    F4 = sbp.tile([P, ATILE, IN_FEATS], bf16, tag="F4")
                    nc.sync.dma_start(
                        out=F4[:, 0:bt, :],
                        in_=src_dram[t0 * P:(t0 + bt) * P, :].rearrange(
                            "(t p) f -> p t f", p=P))
                    mv4 = sbp.tile([P, ATILE, 2], f32, tag="mv4")
                    for i in range(bt):
                        st = sbp.tile([P, 6], f32, tag="st")
                        nc.vector.bn_stats(out=st[:], in_=F4[:, i, :])
                        nc.vector.bn_aggr(out=mv4[:, i, :], in_=st[:])
                    sd4 = sbp.tile([P, ATILE], f32, tag="sd4")
                    nc.scalar.activation(out=sd4[:, 0:bt], in_=mv4[:, 0:bt, 1],
                                         func=AF.Sqrt, bias=eps_c[:])
                    rstd4 = sbp.tile([P, ATILE], f32, tag="rstd4")
                    nc.vector.reciprocal(out=rstd4[:, 0:bt], in_=sd4[:, 0:bt])
                    h4 = sbp.tile([P, ATILE, IN_FEATS], bf16, tag="h4")
                    for i in range(bt):
                        nc.vector.tensor_scalar(
                            out=h4[:, i, :], in0=F4[:, i, :],
                            scalar1=mv4[:, i, 0:1], scalar2=rstd4[:, i:i + 1],
                            op0=Alu.subtract, op1=Alu.mult)
                    hT4 = sbp.tile([P, ATILE, 2, P], bf16, tag="hT4")
                    nc.sync.dma_start_transpose(out=hT4[:, 0:bt, :, :],
                                                in_=h4[:, 0:bt, :])
                    batch = []
                    for i in range(bt):
                        g = psp.tile([P, wcols], f32, tag="gemm")
                        nc.tensor.matmul(out=g[:], lhsT=hT4[:, i, 0, :],
                                         rhs=wtile[:, 0, :], start=True, stop=False)
                        nc.tensor.matmul(out=g[:], lhsT=hT4[:, i, 1, :],
                                         rhs=wtile[:, 1, :], start=False, stop=True)
                        emit(t0 + i, i, g, mv4, rstd4, sbp, batch)
                    if batch:
                        st4, bt0 = batch[0]
                        nc.sync.dma_start(
                            out=fsv_t[bt0 * P:(bt0 + bt) * P, :].rearrange(
                                "(t p) f -> p t f", p=P),
                            in_=st4[:, 0:bt, :])

        def emit_fsv(t, i, g, mv4, rstd4, sbp, batch):
            if i == 0:
                st4 = sbp.tile([P, ATILE, TBL_COLS], bf16, tag="fsv4")
                batch.append((st4, t))
            st4, _ = batch[0]
            nc.vector.tensor_copy(out=st4[:, i, 0:OUT_FEATS],
                                  in_=g[:, 0:OUT_FEATS])
            nc.scalar.activation(out=st4[:, i, OUT_FEATS:], in_=g[:, OUT_FEATS:],
                                 func=AF.Copy)

        def emit_fd(t, i, g, mv4, rstd4, sbp, batch):
            nc.vector.tensor_copy(out=fd_slice[:, t, 0:P], in_=g[:, 0:P])
            nc.scalar.activation(out=fd_slice[:, t, P:OUT_FEATS], in_=g[:, P:],
                                 func=AF.Copy)
            nc.vector.tensor_copy(out=stats_my[:, t, 0:1], in_=mv4[:, i, 0:1])
            nc.vector.tensor_copy(out=stats_my[:, t, 1:2],
                                  in_=rstd4[:, i:i + 1])

        node_sweep(featmy_t, NWIN, emit_fd, wfd, OUT_FEATS)
        node_sweep(featb_t, NTILES, emit_fsv, wfsv, TBL_COLS)

        # ---------------- Phase B: edge phase ----------------
        fsv_hi = fsv_t[HALF:N_PAD, :]
        import os as _os
        _bb = lambda k, d: int(_os.environ.get(k, d))
        with tc.tile_pool(name="b_glo", bufs=_bb("BGL", 3)) as glop, \
             tc.tile_pool(name="b_ghi", bufs=_bb("BGH", 3)) as ghip, \
             tc.tile_pool(name="b_ohg", bufs=_bb("BOG", 3)) as ohgp, \
             tc.tile_pool(name="b_oha", bufs=_bb("BOA", 3)) as ohap, \
             tc.tile_pool(name="b_sev", bufs=_bb("BSV", 4)) as sevp, \
             tc.tile_pool(name="b_e8", bufs=_bb("BE8", 4)) as e8p, \
             tc.tile_pool(name="b_md", bufs=_bb("BMD", 3)) as mdp, \
             tc.tile_pool(name="b_hw", bufs=_bb("BHW", 3)) as hwp, \
             tc.tile_pool(name="b_ev", bufs=_bb("BEV", 3), space="PSUM") as evp, \
             tc.tile_pool(name="b_sc", bufs=_bb("BSC", 3), space="PSUM") as scp, \
             tc.tile_pool(name="b_hu", bufs=_bb("BHU", 2), space="PSUM") as hup:

            icol_of = np.concatenate([[0], np.cumsum([8 * c for c in cpw])])

            def win_gather(w):
                """Issue gathers + oh loads + one-hot builds for window w."""
                cb = cbase[w]
                icol = int(icol_of[w])
                tiles = {}
                ohg_w = ohgp.tile([P, CPWMX, P], bf16, tag="ohg")
                nc.sync.dma_start(out=ohg_w[:, 0:cpw[w], :],
                                  in_=ohg_t[:, cb:cb + cpw[w], :])
                MAXC = 8  # >1024 idx per dma_gather call wedges the device
                if lowC[w]:
                    G = glop.tile([P, LCMX, TBL_COLS], bf16, tag="Glo")
                    o = 0
                    while o < lowC[w]:
                        g = min(MAXC, lowC[w] - o)
                        ni = g * P
                        nc.gpsimd.dma_gather(
                            out_ap=G[:, o:o + g, :], in_ap=fsv_t[:, :],
                            idxs_ap=gidx[:, icol:icol + ni // 16],
                            num_idxs=ni, num_idxs_reg=ni, elem_size=TBL_COLS)
                        icol += ni // 16
                        o += g
                    tiles["lo"] = G
                if highC[w]:
                    G = ghip.tile([P, HCMX, TBL_COLS], bf16, tag="Ghi")
                    o = 0
                    while o < highC[w]:
                        g = min(MAXC, highC[w] - o)
                        ni = g * P
                        nc.gpsimd.dma_gather(
                            out_ap=G[:, o:o + g, :], in_ap=fsv_hi,
                            idxs_ap=gidx[:, icol:icol + ni // 16],
                            num_idxs=ni, num_idxs_reg=ni, elem_size=TBL_COLS)
                        icol += ni // 16
                        o += g
                    tiles["hi"] = G
                oha_w = ohap.tile([P, CPWMX, P], bf16, tag="oha")
                for c in range(cpw[w]):
                    nc.vector.tensor_scalar(
                        out=oha_w[:, c, :], in0=iotab[:],
                        scalar1=dstf[:, cb + c:cb + c + 1], scalar2=None,
                        op0=Alu.is_equal)
                return tiles, ohg_w, oha_w

            def chunk_of(w, c):
                """(section G-key, local idx) for chunk c of window w."""
                if c < lowC[w]:
                    return "lo", c
                return "hi", c - lowC[w]

            def win_scores(w, tiles, ohg_w):
                """evT + silu + score matmuls; returns score psum tile."""
                score_ps = scp.tile([P, CPWMX, NUM_HEADS], f32, tag="score")
                for c0 in range(0, cpw[w], 2):
                    gn = min(2, cpw[w] - c0)
                    ev2 = evp.tile([P, 2, 2, P], f32, tag="ev2")
                    for i in range(gn):
                        sec, lc = chunk_of(w, c0 + i)
                        G = tiles[sec]
                        for b in range(2):
                            nc.tensor.matmul(
                                out=ev2[:, i, b, :],
                                lhsT=fd_slice[:, w, b * P:(b + 1) * P],
                                rhs=ohg_w[:, c0 + i, :], start=True, stop=False)
                            nc.tensor.matmul(
                                out=ev2[:, i, b, :],
                                lhsT=G[:, lc, b * P:(b + 1) * P],
                                rhs=identb[:], start=False, stop=True)
                    sevT = sevp.tile([P, 2, 2, P], bf16, tag="sevT")
                    nc.scalar.activation(out=sevT[:, 0:gn, :, :],
                                         in_=ev2[:, 0:gn, :, :], func=AF.Silu)
                    for i in range(gn):
                        for b in range(2):
                            nc.tensor.matmul(
                                out=score_ps[:, c0 + i, :],
                                lhsT=sevT[:, i, b, :], rhs=attnb[:, b, :],
                                start=(b == 0), stop=(b == 1))
                return score_ps

            def win_finish(w, tiles, ohg_w, oha_w, score_ps):
                e8_w = e8p.tile([P, CPWMX, NUM_HEADS], bf16, tag="e8")
                nc.scalar.activation(out=e8_w[:, 0:cpw[w], :],
                                     in_=score_ps[:, 0:cpw[w], :], func=AF.Exp)
                mds = {}
                for sec, g, off in (("lo", lowC[w], 0), ("hi", highC[w], lowC[w])):
                    if not g:
                        continue
                    G = tiles[sec]
                    md = mdp.tile([P, LCMX if sec == "lo" else HCMX, OUT_FEATS],
                                  bf16, tag="md" + sec)
                    nc.vector.tensor_tensor(
                        out=md[:, 0:g, :].rearrange("p c (f h) -> p c f h",
                                                    h=NUM_HEADS),
                        in0=G[:, 0:g, OUT_FEATS:].rearrange(
                            "p c (f h) -> p c f h", h=NUM_HEADS),
                        in1=e8_w[:, off:off + g, None, :].to_broadcast(
                            [P, g, FPH, NUM_HEADS]),
                        op=Alu.mult)
                    mds[sec] = md
                # note: matmul accumulation groups must stay contiguous per
                # PSUM region on HW - interleaving two regions corrupts one.
                hupd = hup.tile([P, AGG_COLS], f32, tag="hupd")
                for c in range(cpw[w]):
                    sec, lc = chunk_of(w, c)
                    nc.tensor.matmul(out=hupd[:, 0:OUT_FEATS],
                                     lhsT=oha_w[:, c, :], rhs=mds[sec][:, lc, :],
                                     start=(c == 0), stop=(c == cpw[w] - 1))
                for c in range(cpw[w]):
                    nc.tensor.matmul(out=hupd[:, OUT_FEATS:],
                                     lhsT=oha_w[:, c, :], rhs=e8_w[:, c, :],
                                     start=(c == 0), stop=(c == cpw[w] - 1))
                hw_s = hwp.tile([P, AGG_COLS], bf16, tag="hw_s")
                nc.vector.tensor_copy(out=hw_s[:], in_=hupd[:])
                nc.sync.dma_start(out=hupd_t[w], in_=hw_s[:])

            w = 0
            while w < NWIN:
                pair = [w] if w + 1 >= NWIN else [w, w + 1]
                state = []
                for ww in pair:
                    tiles, ohg_w, oha_w = win_gather(ww)
                    score_ps = win_scores(ww, tiles, ohg_w)
                    state.append((ww, tiles, ohg_w, oha_w, score_ps))
                for (ww, tiles, ohg_w, oha_w, score_ps) in state:
                    win_finish(ww, tiles, ohg_w, oha_w, score_ps)
                w += len(pair)

        # ---------------- Phase C: normalize + residual + silu ----------------
        with tc.tile_pool(name="c_sb", bufs=3) as sbp:
            for w0 in range(0, NWIN, ATILE):
                bt = min(ATILE, NWIN - w0)
                hu4 = sbp.tile([P, ATILE, AGG_COLS], bf16, tag="hu4")
                nc.sync.dma_start(out=hu4[:, 0:bt, :],
                                  in_=hupd_t[w0:w0 + bt].rearrange(
                                      "w p c -> p w c"))
                F4 = sbp.tile([P, ATILE, IN_FEATS], bf16, tag="F4")
                nc.sync.dma_start(
                    out=F4[:, 0:bt, :],
                    in_=featmy_t[w0 * P:(w0 + bt) * P, :].rearrange(
                        "(t p) f -> p t f", p=P))
                for i in range(bt):
                    w = w0 + i
                    h = sbp.tile([P, IN_FEATS], bf16, tag="h")
                    nc.vector.tensor_scalar(
                        out=h[:], in0=F4[:, i, :], scalar1=stats_my[:, w, 0:1],
                        scalar2=stats_my[:, w, 1:2],
                        op0=Alu.subtract, op1=Alu.mult)
                    den = sbp.tile([P, NUM_HEADS], f32, tag="den")
                    nc.vector.tensor_scalar_add(out=den[:],
                                                in0=hu4[:, i, OUT_FEATS:],
                                                scalar1=1e-30)
                    denr = sbp.tile([P, NUM_HEADS], f32, tag="denr")
                    nc.vector.reciprocal(out=denr[:], in_=den[:])
                    o = sbp.tile([P, FPH, NUM_HEADS], f32, tag="o")
                    nc.vector.tensor_tensor(
                        out=o[:],
                        in0=hu4[:, i, 0:OUT_FEATS].rearrange(
                            "p (f h) -> p f h", h=NUM_HEADS),
                        in1=denr[:, None, :].to_broadcast([P, FPH, NUM_HEADS]),
                        op=Alu.mult)
                    on = sbp.tile([P, OUT_FEATS], f32, tag="on")
                    nc.vector.tensor_tensor(
                        out=on[:].rearrange("p (h f) -> p h f", h=NUM_HEADS),
                        in0=o[:].rearrange("p f h -> p h f"),
                        in1=h[:].rearrange("p (h f) -> p h f", h=NUM_HEADS),
                        op=Alu.add)
                    oo = sbp.tile([P, OUT_FEATS], f32, tag="oo")
                    nc.scalar.activation(out=oo[:], in_=on[:], func=AF.Silu)
                    nc.sync.dma_start(out=out_t[w * P:(w + 1) * P, :], in_=oo[:])

    nc.compile()
    return nc, (lowC, highC)


def _derive_schedule(src, dst):
    """Per-window chunk counts (low/high table half), maxed over cores."""
    lowC = np.zeros(NWIN, np.int64)
    highC = np.zeros(NWIN, np.int64)
    for core in range(N_CORES):
        lo, hi = core * SLICE, (core + 1) * SLICE
        m = (dst >= lo) & (dst < hi)
        w_of = (dst[m] - lo) // P
        is_lo = src[m] < HALF
        cl = np.bincount(w_of[is_lo], minlength=NWIN)
        ch = np.bincount(w_of[~is_lo], minlength=NWIN)
        lowC = np.maximum(lowC, (cl + P - 1) // P)
        highC = np.maximum(highC, (ch + P - 1) // P)
    return tuple(int(x) for x in lowC), tuple(int(x) for x in highC)


def _prepare_core_inputs(core, src, dst, lowC, highC):
    import ml_dtypes
    cpw = [l + h for l, h in zip(lowC, highC)]
    C_TOT = sum(cpw)
    cbase = np.concatenate([[0], np.cumsum(cpw)]).astype(int)

    lo, hi = core * SLICE, (core + 1) * SLICE
    m = (dst >= lo) & (dst < hi)
    dsl = dst[m] - lo
    ssl = src[m]
    w_of = dsl // P
    is_lo = ssl < HALF

    slot_src = np.zeros((C_TOT, P), np.int64)
    slot_doff = np.full((C_TOT, P), -1.0, np.float32)

    for w in range(NWIN):
        wm = w_of == w
        for half, cb, g in ((True, cbase[w], lowC[w]),
                            (False, cbase[w] + lowC[w], highC[w])):
            sel = wm & (is_lo == half)
            s_w = ssl[sel]
            d_w = dsl[sel] - w * P
            n = len(s_w)
            assert n <= g * P, (core, w, half, n, g * P)
            flat_s = slot_src[cb:cb + g].reshape(-1)
            flat_d = slot_doff[cb:cb + g].reshape(-1)
            flat_s[:n] = s_w
            flat_d[:n] = d_w

    dstf = slot_doff.T.copy()  # [P, C_TOT] f32

    ohg = np.zeros((P, C_TOT, P), ml_dtypes.bfloat16)
    cc, ee = np.nonzero(slot_doff >= 0)
    ohg[slot_doff[cc, ee].astype(np.int64), cc, ee] = 1

    idx_cols = []
    for w in range(NWIN):
        for half, cb, g in ((True, cbase[w], lowC[w]),
                            (False, cbase[w] + lowC[w], highC[w])):
            if not g:
                continue
            s = slot_src[cb:cb + g].reshape(-1).copy()
            if not half:
                s = np.maximum(s - HALF, 0)
            idx = s.astype(np.int16)
            idx_cols.append(np.tile(idx.reshape(-1, 16).T, (8, 1)))
    gidx = np.concatenate(idx_cols, axis=1).astype(np.int16)
    return dstf, gidx, ohg


def _shared_inputs(inputs):
    import ml_dtypes
    feat = np.asarray(inputs["feat"], np.float32)
    Wsrc = np.asarray(inputs["Wsrc"], np.float32)
    Wdst = np.asarray(inputs["Wdst"], np.float32)
    Wval = np.asarray(inputs["Wval"], np.float32)
    attn = np.asarray(inputs["attn"], np.float32).reshape(NUM_HEADS, FPH)

    featb = np.zeros((N_PAD, IN_FEATS), ml_dtypes.bfloat16)
    featb[:N_NODES] = feat.astype(ml_dtypes.bfloat16)

    # weights transposed + output-column permuted to (f,h)-major
    WsrcP = Wsrc[_OLD_OF_NEW, :]   # [256 newcol, 256 in]
    WvalP = Wval[_OLD_OF_NEW, :]
    WdstP = Wdst[_OLD_OF_NEW, :]
    wfsv = np.zeros((P, 2, TBL_COLS), np.float32)
    for b in range(2):
        wfsv[:, b, 0:OUT_FEATS] = WsrcP[:, b * P:(b + 1) * P].T
        wfsv[:, b, OUT_FEATS:] = WvalP[:, b * P:(b + 1) * P].T
    wfd = np.zeros((P, 2, OUT_FEATS), np.float32)
    for b in range(2):
        wfd[:, b, :] = WdstP[:, b * P:(b + 1) * P].T

    attnb = np.zeros((P, 2, NUM_HEADS), np.float32)
    for b in range(2):
        j = b * P + np.arange(P)
        attnb[np.arange(P), b, j % NUM_HEADS] = attn[j % NUM_HEADS,
                                                     j // NUM_HEADS]

    identb = np.eye(P, dtype=ml_dtypes.bfloat16)
    iotab = np.tile(np.arange(P, dtype=ml_dtypes.bfloat16).reshape(1, P),
                    (P, 1))
    bf = ml_dtypes.bfloat16
    return (featb, feat, wfsv.astype(bf), wfd.astype(bf), attnb.astype(bf),
            identb, iotab)


def make_in_maps(inputs, lowC, highC):
    import ml_dtypes
    featb, feat, wfsv, wfd, attnb, identb, iotab = _shared_inputs(inputs)
    src = np.asarray(inputs["src"], np.int64)
    dst = np.asarray(inputs["dst"], np.int64)
    in_maps = []
    for core in range(N_CORES):
        dstf, gidx, ohg = _prepare_core_inputs(core, src, dst, lowC, highC)
        featmy = np.zeros((SLICE_PAD, IN_FEATS), ml_dtypes.bfloat16)
        n = min(SLICE_PAD, N_NODES - core * SLICE)
        featmy[:n] = feat[core * SLICE:core * SLICE + n].astype(
            ml_dtypes.bfloat16)
        in_maps.append(dict(
            featb=featb, featmy=featmy, wfsv=wfsv, wfd=wfd, attnb=attnb,
            identb=identb, iotab=iotab, dstf=dstf, gidx=gidx, ohg=ohg,
        ))
    return in_maps


def kernel(**inputs):
    import concourse.bass_utils as bass_utils

    for b in ("bsrc", "bdst", "bval"):
        assert not np.any(np.asarray(inputs[b])), \
            "nonzero biases unsupported by this kernel"
    src = np.asarray(inputs["src"], np.int64)
    dst = np.asarray(inputs["dst"], np.int64)

    lowC, highC = _derive_schedule(src, dst)
    key = (lowC, highC)
    if key not in _CACHE:
        _CACHE[key] = _build_nc(lowC, highC)
    nc, _ = _CACHE[key]

    in_maps = make_in_maps(inputs, lowC, highC)
    res = bass_utils.run_bass_kernel_spmd(nc, in_maps, core_ids=list(range(N_CORES)))
    out = np.concatenate(
        [res.results[c]["outmy"][:SLICE] for c in range(N_CORES)], axis=0)
    return np.ascontiguousarray(out.astype(np.float32))
